# revision 11
# baseline (speedup 1.0000x reference)
"""CQAttention Trainium2 kernel v3 — [d,c] A/Bt, raw export, host norm.

Device per batch (D=128, Lc=1024, Lq=128):
    e1[q,c]    = exp(s_m^T + s_q - ln16)     ACT, bias = sq16 col of In16
    e2[c,q]    = exp(s_m)   (8 c-tiles)      ACT
    W2[q,d]    = (e2^T @ [Ct*esc | esc])/z2  8 accum MMs + recip + act
    At[d,c]    = qt^T @ e1                   2 MMs N=512   (= (S1raw/16 @ Qt)^T)
    Bt[d,c]    = w2^T @ e1                   2 MMs N=512
    z[c]=z1/16 = e1_t^T @ 1                  8 tiny MMs (e1-tile stationaries)
Exports fp16 [d, {At 1024 | Bt 1024 | z 8}]; host divides by z (scales
cancel) and assembles [Ct, A, Ct*A, Ct*Bt] in fp32.

PSUM (8 banks): pB bufs=1 x 2 banks (t1 = S^T, one merged e1 exp);
pA bufs=2 x 1 bank (t2a, t2b = S-tile quads, t3 = W2-accum + z cols);
tab bufs=1 x 4 banks (At|Bt), drained by ONE DVE copy.  w2 normalization
on DVE (tensor_scalar_mul), z on DVE; ACT runs only 3 exp/copy ops.
Emission: st/e1 first so the merged e1 starts early; atbt of batch k-2
between, keeping every engine queue deadlock-free with zero-stall rings.
"""

import warnings

warnings.filterwarnings("ignore")

import numpy as np

B, D, LC, LQ = 64, 128, 1024, 128
NT = 8
NCORES = 8
NB = B // NCORES
NEG16 = -30000.0
NIN = LC + LQ + NT * 129 + LQ + 1  # cb | qww | rhsB | qt | sq16
RB0 = LC + LQ
QT0 = RB0 + NT * 129
SQ0 = QT0 + LQ
NOUT = 2 * LC + NT  # At | Bt | z
LN16 = float(np.log(16.0))

CFG = {"v": 12}

_CACHE = {}


def _build_nc(reps=1):
    import concourse.bass as bass
    import concourse.mybir as mybir
    import concourse.tile as tile
    from concourse import bacc

    F32 = mybir.dt.float32
    F16 = mybir.dt.float16
    AF = mybir.ActivationFunctionType

    nc = bacc.Bacc("TRN2", target_bir_lowering=False, debug=False,
                   num_devices=NCORES)

    In16 = nc.dram_tensor("In16", [NB, D, NIN], F16, kind="ExternalInput")
    Out = nc.dram_tensor("Out", [NB, D, NOUT], F16, kind="ExternalOutput")

    with tile.TileContext(nc) as tc:
        with tc.tile_pool(name="const", bufs=1) as constp, \
             tc.tile_pool(name="io", bufs=4) as iop, \
             tc.tile_pool(name="sb", bufs=4) as sb, \
             tc.tile_pool(name="sm", bufs=4) as sm, \
             tc.tile_pool(name="pa", bufs=2, space="PSUM") as pA, \
             tc.tile_pool(name="pb", bufs=1, space="PSUM") as pB, \
             tc.tile_pool(name="tat", bufs=1, space="PSUM") as tatp, \
             tc.tile_pool(name="tbt", bufs=1, space="PSUM") as tbtp:

            ones1 = constp.tile([D, 1], F16)
            nc.gpsimd.memset(ones1[:], 1.0)
            warm = constp.tile([D, 1], F16)
            nc.scalar.activation(warm[:], ones1[:], AF.Exp)

            def batch_phases(b):
                st = {}

                def ph_load():
                    inb = iop.tile([D, NIN], F16, tag="inb", name=f"inb{b}")
                    st["inb"] = inb
                    nc.sync.dma_start(inb[:], In16[b])

                def ph_load_split():
                    inb = iop.tile([D, NIN], F16, tag="inb", name=f"inb{b}")
                    st["inb"] = inb
                    nc.sync.dma_start(inb[:, 0:RB0], In16[b][:, 0:RB0])
                    nc.sync.dma_start(inb[:, RB0:NIN], In16[b][:, RB0:NIN])

                def ph_s0():
                    inb = st["inb"]
                    cb = inb[:, 0:LC]
                    qww = inb[:, LC:LC + LQ]
                    for h in range(2):
                        t2 = pA.tile([D, 4, D], F32, tag="pa",
                                     name=f"t2_{b}_{h}")
                        st[f"t2{h}"] = t2
                        for i in range(4):
                            ct = 4 * h + i
                            nc.tensor.matmul(t2[:, i, :],
                                             cb[:, ct * D:(ct + 1) * D],
                                             qww[:], start=True, stop=True)

                def ph_e2():
                    e2 = sb.tile([D, NT, D], F16, tag="e2", name=f"e2_{b}")
                    st["e2"] = e2
                    for h in range(2):
                        nc.scalar.activation(e2[:, 4 * h:4 * h + 4, :],
                                             st[f"t2{h}"][:], AF.Exp)

                def ph_st():
                    inb = st["inb"]
                    cb = inb[:, 0:LC]
                    qww = inb[:, LC:LC + LQ]
                    t1 = pB.tile([D, LC], F32, tag="pb", name=f"t1_{b}")
                    st["t1"] = t1
                    for h in range(2):
                        nc.tensor.matmul(t1[:, 512 * h:512 * (h + 1)],
                                         qww[:],
                                         cb[:, 512 * h:512 * (h + 1)],
                                         start=True, stop=True)

                def ph_e1():
                    inb = st["inb"]
                    sq = inb[:, SQ0:SQ0 + 1]
                    e1 = sb.tile([D, LC], F16, tag="e1", name=f"e1_{b}")
                    st["e1"] = e1
                    nc.scalar.activation(e1[:], st["t1"][:], AF.Exp, bias=sq)

                def ph_w2():
                    inb, e2 = st["inb"], st["e2"]
                    rb = inb[:, RB0:QT0].rearrange("p (t j) -> p t j", t=NT)
                    t3 = pA.tile([D, 512], F32, tag="pa", name=f"t3_{b}")
                    st["t3"] = t3
                    for t in range(NT):
                        nc.tensor.matmul(t3[:, 0:129], e2[:, t, :],
                                         rb[:, t, :],
                                         start=(t == 0), stop=(t == NT - 1))
                    r2 = sm.tile([D, 1], F32, tag="r2", name=f"r2_{b}")
                    nc.vector.reciprocal(r2[:], t3[:, 128:129])
                    w2 = sb.tile([D, D], F16, tag="w2", name=f"w2_{b}")
                    st["w2"] = w2
                    nc.vector.tensor_scalar_mul(w2[:], t3[:, 0:128], r2[:])

                def ph_atbt():
                    inb, e1, w2, t3 = st["inb"], st["e1"], st["w2"], st["t3"]
                    qt = inb[:, QT0:QT0 + LQ]
                    tat = tatp.tile([D, LC], F32, tag="tat", name=f"tat{b}")
                    tbt = tbtp.tile([D, LC], F32, tag="tbt", name=f"tbt{b}")
                    st["tat"], st["tbt"] = tat, tbt
                    ob = iop.tile([D, NOUT], F16, tag="ob", name=f"ob{b}")
                    st["ob"] = ob
                    for h in range(2):
                        nc.tensor.matmul(tat[:, 512 * h:512 * (h + 1)],
                                         qt[:],
                                         e1[:, 512 * h:512 * (h + 1)],
                                         start=True, stop=True)
                    for t in range(NT):
                        nc.tensor.matmul(t3[:, 384 + t:385 + t],
                                         e1[:, t * D:(t + 1) * D],
                                         ones1[:], start=True, stop=True)
                    for h in range(2):
                        nc.tensor.matmul(tbt[:, 512 * h:512 * (h + 1)],
                                         w2[:],
                                         e1[:, 512 * h:512 * (h + 1)],
                                         start=True, stop=True)

                def ph_zfin():
                    nc.vector.tensor_copy(st["ob"][:, 2 * LC:2 * LC + NT],
                                          st["t3"][:, 384:392])

                def ph_atcopy():
                    nc.vector.tensor_copy(st["ob"][:, 0:LC], st["tat"][:])

                def ph_btcopy():
                    nc.vector.tensor_copy(st["ob"][:, LC:2 * LC], st["tbt"][:])

                def ph_btcopy_act():
                    nc.scalar.activation(st["ob"][:, LC:2 * LC], st["tbt"][:],
                                         AF.Copy)

                def ph_out():
                    nc.sync.dma_start(Out[b], st["ob"][:])

                def ph_out_a():
                    nc.sync.dma_start(Out[b][:, 0:LC], st["ob"][:, 0:LC])

                def ph_out_b():
                    nc.sync.dma_start(Out[b][:, LC:NOUT],
                                      st["ob"][:, LC:NOUT])

                st["phases"] = dict(
                    load=ph_load, s0=ph_s0, e2=ph_e2, st=ph_st, e1=ph_e1,
                    w2=ph_w2, atbt=ph_atbt, zfin=ph_zfin,
                    atcopy=ph_atcopy, btcopy=ph_btcopy,
                    btcopy_act=ph_btcopy_act, out=ph_out,
                    out_a=ph_out_a, out_b=ph_out_b,
                    load_split=ph_load_split)
                return st

            order = [bb for _ in range(reps) for bb in range(NB)]
            N = len(order)
            P = {}
            for k in range(N + 2):
                if 0 <= k - 1 < N:
                    P[k - 1]["phases"]["st"]()
                    P[k - 1]["phases"]["e1"]()
                if 0 <= k - 2 < N:
                    P[k - 2]["phases"]["atbt"]()
                    P[k - 2]["phases"]["zfin"]()
                    P[k - 2]["phases"]["atcopy"]()
                if 0 <= k - 1 < N:
                    P[k - 1]["phases"]["s0"]()
                    P[k - 1]["phases"]["e2"]()
                    P[k - 1]["phases"]["w2"]()
                if 0 <= k - 2 < N:
                    if k - 2 == N - 1:
                        P[k - 2]["phases"]["btcopy_act"]()
                        P[k - 2]["phases"]["out_a"]()
                    else:
                        P[k - 2]["phases"]["btcopy"]()
                if k < N:
                    P[k] = batch_phases(order[k])
                    P[k]["phases"]["load_split" if k == 0 else "load"]()
                if 0 <= k - 2 < N:
                    if k - 2 == N - 1:
                        P[k - 2]["phases"]["out_b"]()
                    else:
                        P[k - 2]["phases"]["out"]()

    nc.compile()
    return nc


def _prep_inmaps(C, Q, cmask, qmask, w):
    C64 = np.asarray(C, np.float64)
    Q64 = np.asarray(Q, np.float64)
    w64 = np.asarray(w, np.float64)
    wq, wc, wm = w64[:D], w64[D:2 * D], w64[2 * D:]
    C16 = C64.astype(np.float16)                          # [B, D, LC]
    qww16 = (Q64 * wm[None, :, None]).astype(np.float16)  # [B, D, LQ]
    qbias = (1.0 - np.asarray(qmask, np.float64)) * NEG16
    cbias = (1.0 - np.asarray(cmask, np.float64)) * NEG16
    sqf16 = np.ascontiguousarray(
        (np.einsum("bdq,d->bq", Q64, wq) + qbias - LN16)
        .astype(np.float16)[:, :, None])                  # [B, q, 1]
    scb = np.einsum("bdc,d->bc", C64, wc) + cbias         # [B, LC]
    esc = np.exp(scb)                                     # [B, LC]
    ctesc = C64 * esc[:, None, :]                         # [B, D, LC]
    rb = np.empty((B, D, NT, 129), np.float16)
    rb[..., 0:128] = ctesc.reshape(B, D, NT, D).transpose(0, 3, 2, 1)
    rb[..., 128] = esc.reshape(B, NT, D).transpose(0, 2, 1)
    qtb = Q64.astype(np.float16).transpose(0, 2, 1)       # [B, q, d]
    in16 = np.concatenate(
        [C16, qww16, rb.reshape(B, D, NT * 129), qtb, sqf16], axis=2)
    in16 = np.ascontiguousarray(in16)
    in_maps = []
    for k in range(NCORES):
        s = slice(k * NB, (k + 1) * NB)
        in_maps.append({"In16": in16[s]})
    return in_maps


def _run(C, Q, cmask, qmask, w, trace=False):
    from concourse.bass_utils import run_bass_kernel_spmd

    key = (tuple(sorted(CFG.items())), 1)
    if key not in _CACHE:
        _CACHE[key] = _build_nc()
    nc = _CACHE[key]
    in_maps = _prep_inmaps(C, Q, cmask, qmask, w)
    res = run_bass_kernel_spmd(nc, in_maps, core_ids=list(range(NCORES)),
                               trace=trace)
    dev = np.concatenate([res.results[k]["Out"] for k in range(NCORES)],
                         axis=0)                          # [B, D, NOUT]
    return dev, res


def _assemble(dev, C):
    """dev: [n, D, 2*LC+NT] fp16; C: [n, D, LC] input."""
    n = dev.shape[0]
    dv = np.asarray(dev, np.float32)
    z = dv[:, :, 2 * LC:]                                 # [n, j, t] = z1/16
    r1 = (1.0 / z).transpose(0, 2, 1).reshape(n, 1, LC)   # [n, 1, c]
    A = dv[:, :, 0:LC] * r1                               # [n, d, c]
    Bt = dv[:, :, LC:2 * LC] * r1
    Cf = np.asarray(C, np.float32)
    out = np.empty((n, 4 * D, LC), np.float32)
    out[:, 0:D] = Cf
    out[:, D:2 * D] = A
    out[:, 2 * D:3 * D] = Cf * A
    out[:, 3 * D:4 * D] = Cf * Bt
    return out


def kernel(C, Q, cmask, qmask, w):
    dev, _ = _run(C, Q, cmask, qmask, w, trace=False)
    return _assemble(dev, C)
